# revision 48
# baseline (speedup 1.0000x reference)
"""8-core Trainium2 (Bass/Tile) kernel for nn_CrossAttention.

Sharding: pure data parallelism - batch B=8, one batch element per
NeuronCore. Each core runs QKV projections, 16-head attention with
relative position bias, and the output projection for its element;
the host gathers the 8 outputs.

Host-side prep (cheap numpy transforms of the inputs, same category as
the exp(rpb) bias transform):
  - LayerNorm of x/context on host (fp32), shipped TRANSPOSED and
    decomposed into fp8e4 hi+lo pairs (hi = fp8(xn), lo = fp8(xn-hi)).
  - gamma folded into W{q,k,v}; weights scaled by 32 (so their fp8 lo
    parts stay above the fp8 subnormal cutoff) and decomposed hi+lo.
    The 32x cancels: scores get SM_SCALE/1024 inside the exp, and the
    attn@v rowsum column uses 32.0 instead of 1.0.
  - relative_position_bias shipped as exp(b)^T in fp16.
  - beta@W biases scaled by 32 (fp32), Wo in fp16.

Q/K/V projections run as compensated fp8 DoubleRow matmuls: three
passes (hi*hi + hi*lo + lo*hi) of K=256 contractions at 0.5 cycles per
output row - 25% fewer charged PE rows than fp16 at full precision
(the dropped lo*lo term is ~0.04%).

attn@v is "flipped": lhsT = eh tile ([128 k, 128 q]), rhs = per-head V
([128 k, 64 d]) -> psum [128 q, 64 d] accumulated over 8 kt steps, so
per head it streams 4k rows instead of 8k. Rowsums come from extra N=1
matmuls against the 32.0 column; normalization is an fp32 reciprocal +
stride-0 broadcast DVE multiply; PE transposes restore [hd, q] for the
fp16 output projection.

The head loop is software-pipelined at kt-slot granularity: each slot
is one scores matmul pair plus filler PE work (next chunk's k/q
projection subunits, the trailing head's attn@v parts, ao transposes)
so the PE tracks the ACT exp pace (~1.04us per [128,1024] tile).

PSUM (8 banks): scores 2x[128,1024]f32 (4), k/q proj halves 2x1,
attnv data+rowsums [128,1024]f32 (2).
"""

import numpy as np
import ml_dtypes

import concourse.bass as bass
import concourse.bacc as bacc
import concourse.tile as tile
from concourse import mybir
from concourse.masks import make_identity
from concourse.bass_utils import run_bass_kernel_spmd

f32 = mybir.dt.float32
f16 = mybir.dt.float16
f8 = mybir.dt.float8e4
AF = mybir.ActivationFunctionType
ALU = mybir.AluOpType
DR = mybir.MatmulPerfMode.DoubleRow

N = 1024
D = 1024
H = 16
NT = 8
KC = 8
EPS = 1e-5
WS = 32.0                      # host weight scale
SM_SCALE = 0.125 / (WS * WS)   # exp(SM_SCALE * q'k'), q'k' = 1024 qk
N_CORES = 8
F8NP = ml_dtypes.float8_e4m3fn


def _body(tc, nc, ins, out_d):
    with (
        tc.tile_pool(name="consts", bufs=1) as consts,
        tc.tile_pool(name="big", bufs=1) as big,
        tc.tile_pool(name="pacc", bufs=2, space="PSUM") as pacc,
        tc.tile_pool(name="pproj", bufs=2, space="PSUM") as pproj,
        tc.tile_pool(name="pas", bufs=1, space="PSUM") as pas_pool,
    ):
        ident = consts.tile([128, 128], f16)
        make_identity(nc, ident[:])
        eps_t = consts.tile([128, 1], f32)
        nc.vector.memset(eps_t[:], EPS)
        scratch = consts.tile([128, 1], f32)
        bq_t = consts.tile([128, KC], f32)
        bk_t = consts.tile([128, KC], f32)
        bv_ap = ins["bqkv"][2, :]
        bv_b = consts.tile([128, D], f16)
        nc.gpsimd.dma_start(out=bv_b[:], in_=bass.AP(
            tensor=bv_ap.tensor, offset=bv_ap.offset,
            ap=[[0, 128]] + list(bv_ap.ap)))
        bo_ap = ins["bo"][:]
        bo_b = consts.tile([128, D], f16)

        v_aug = big.tile([128, NT, H, 65], f16)
        aoT = big.tile([128, KC, N], f16)
        x8h = big.tile([128, KC, N], f8)
        x8l = big.tile([128, KC, N], f8)
        c8h = big.tile([128, KC, N], f8)
        c8l = big.tile([128, KC, N], f8)
        wq8h = big.tile([128, KC, D], f8)
        wq8l = big.tile([128, KC, D], f8)
        wk8h = big.tile([128, KC, D], f8)
        wk8l = big.tile([128, KC, D], f8)
        wo16 = big.tile([128, KC, D], f16)
        nc.gpsimd.memset(v_aug[:, :, :, 0:1], WS)
        v_flat = v_aug[:].rearrange("p a h c -> p a (h c)")

        def dma_w(dst, name):
            nc.sync.dma_start(out=dst[:], in_=ins[name].rearrange(
                "(a p) m -> p a m", p=128))

        ebh_tiles = {}

        def emit_ebh(h, pebt):
            if h >= H:
                return
            for half in range(2):
                t = pebt.tile([128, 4, N], f16, tag="ebt")
                nc.sync.dma_start(
                    out=t[:],
                    in_=ins["ebt"][h, half * 512:(half + 1) * 512, :].rearrange(
                        "(a p) q -> p a q", p=128))
                ebh_tiles[(h, half)] = t

        # ---------------- prologue ----------------
        with tc.tile_pool(name="pwv", bufs=1) as pwv:
            wv8h = pwv.tile([128, KC, D], f8)
            wv8l = pwv.tile([128, KC, D], f8)
            # DMA issue order: feed vproj's hh pass first, then the rest
            dma_w(c8h, "c8h")
            dma_w(wv8h, "wv8h")
            dma_w(c8l, "c8l")
            dma_w(wv8l, "wv8l")
            nc.sync.dma_start(out=bq_t[:], in_=ins["bqkv"][0, :].rearrange(
                "(m p) -> p m", p=128))
            nc.sync.dma_start(out=bk_t[:], in_=ins["bqkv"][1, :].rearrange(
                "(m p) -> p m", p=128))
            dma_w(wk8h, "wk8h")
            dma_w(wk8l, "wk8l")
            dma_w(x8h, "x8h")
            dma_w(x8l, "x8l")
            dma_w(wq8h, "wq8h")
            dma_w(wq8l, "wq8l")

            def emit_vproj(t):
                pv = pacc.tile([128, N], f32, tag="acc")
                for nq in range(4):
                    idx = 0
                    for (cs, ws) in ((c8h, wv8h), (c8l, wv8h), (c8h, wv8l)):
                        for a in range(4):
                            nc.tensor.matmul(
                                pv[:, nq * 256:(nq + 1) * 256],
                                cs[:, 2 * a:2 * a + 2, t * 128:(t + 1) * 128],
                                ws[:, 2 * a:2 * a + 2,
                                   nq * 256:(nq + 1) * 256],
                                start=(idx == 0), stop=(idx == 11),
                                perf_mode=DR)
                            idx += 1
                nc.vector.tensor_add(
                    out=v_aug[:, t, :, 1:65],
                    in0=pv[:].rearrange("p (h d) -> p h d", h=H),
                    in1=bv_b[:].rearrange("p (h d) -> p h d", h=H))

            for t in range(NT):
                emit_vproj(t)

        with tc.tile_pool(name="pkq", bufs=4) as pkq, \
             tc.tile_pool(name="pc", bufs=3) as pc, \
             tc.tile_pool(name="pebt", bufs=3) as pebt, \
             tc.tile_pool(name="pao", bufs=2) as pao, \
             tc.tile_pool(name="prec", bufs=2) as prec:

            kq_tiles = {}
            WX = {"k": (wk8h, wk8l, c8h, c8l), "q": (wq8h, wq8l, x8h, x8l)}

            def proj_sub(which, c, half, nq, sub):
                """6 of the 12 DoubleRow matmuls of one 256-tok quarter."""
                key = (which, c, half)
                if key not in kq_tiles:
                    kq_tiles[key] = pproj.tile([128, 512], f32, tag="proj",
                                               name=f"p{which}{c}h{half}")
                ph = kq_tiles[key]
                wh, wl, xh, xl = WX[which]
                mms = [(w, x, a) for (w, x) in ((wh, xh), (wh, xl), (wl, xh))
                       for a in range(4)]
                tok0 = half * 512 + nq * 256
                for i in range(6 * sub, 6 * sub + 6):
                    w, x, a = mms[i]
                    nc.tensor.matmul(
                        ph[:, nq * 256:(nq + 1) * 256],
                        w[:, 2 * a:2 * a + 2, c * 128:(c + 1) * 128],
                        x[:, 2 * a:2 * a + 2, tok0:tok0 + 256],
                        start=(i == 0), stop=(i == 11), perf_mode=DR)

            def proj_bias(which, c, half, eng="dve"):
                ph = kq_tiles.pop((which, c, half))
                dkey = (which, c)
                if dkey not in kq_tiles:
                    kq_tiles[dkey] = pkq.tile([128, N], f16, tag="kq",
                                              name=f"{which}T{c}")
                bias = bk_t if which == "k" else bq_t
                dst = kq_tiles[dkey][:, half * 512:(half + 1) * 512]
                if eng == "act":
                    nc.scalar.add(out=dst, in_=ph[:], add=bias[:, c:c + 1])
                else:
                    nc.vector.tensor_scalar(
                        out=dst, in0=ph[:], scalar1=bias[:, c:c + 1],
                        scalar2=None, op0=ALU.add)

            def make_proj_units(which, c, bias_eng="dve"):
                units = []
                for half in range(2):
                    for nq in range(2):
                        for sub in range(2):
                            def u(which=which, c=c, half=half, nq=nq,
                                  sub=sub):
                                proj_sub(which, c, half, nq, sub)
                                if nq == 1 and sub == 1:
                                    proj_bias(which, c, half, bias_eng)
                            units.append(u)
                return units

            def scores_slot(h, kt, eh_t):
                """One kt tile of scores + exp + ebt multiply."""
                ch, r0 = h // 2, (h % 2) * 64
                kTc = kq_tiles[("k", ch)]
                qTc = kq_tiles[("q", ch)]
                ps = pacc.tile([128, N], f32, tag="acc")
                for nh in range(2):
                    nc.tensor.matmul(
                        ps[:, nh * 512:(nh + 1) * 512],
                        kTc[r0:r0 + 64, kt * 128:(kt + 1) * 128],
                        qTc[r0:r0 + 64, nh * 512:(nh + 1) * 512],
                        start=True, stop=True)
                nc.scalar.activation(out=eh_t[:, kt, :], in_=ps[:],
                                     func=AF.Exp, scale=SM_SCALE)
                eng = nc.gpsimd if (kt in (3, 6) and h < H - 2) else nc.vector
                eng.tensor_mul(out=eh_t[:, kt, :], in0=eh_t[:, kt, :],
                               in1=ebh_tiles[(h, kt // 4)][:, kt % 4, :])

            def attnv_part(h, qt, eh_t, pas):
                """attn@v for one qt block of head h: 8 data + 8 sum mms."""
                for kt in range(NT):
                    nc.tensor.matmul(
                        pas[:, qt * 64:(qt + 1) * 64],
                        eh_t[:, kt, qt * 128:(qt + 1) * 128],
                        v_flat[:, kt, h * 65 + 1:h * 65 + 65],
                        start=(kt == 0), stop=(kt == NT - 1))
                for kt in range(NT):
                    nc.tensor.matmul(
                        pas[:, 512 + qt:513 + qt],
                        eh_t[:, kt, qt * 128:(qt + 1) * 128],
                        v_flat[:, kt, h * 65:h * 65 + 1],
                        start=(kt == 0), stop=(kt == NT - 1))

            def emit_norm(h, pas, ao_t):
                rec = prec.tile([128, 8], f32, tag="rec")
                nc.vector.reciprocal(out=rec[:], in_=pas[:, 512:520])
                rec_b = bass.AP(tensor=rec.tensor, offset=rec.offset,
                                ap=[[8, 128], [1, 8], [0, 64]])
                nc.vector.tensor_mul(
                    out=ao_t[:, :, h % 2, :],
                    in0=pas[:, 0:512].rearrange("p (a b) -> p a b", a=8),
                    in1=rec_b)

            def emit_transpose(ch, ao_t):
                ptr = pproj.tile([128, NT, 128], f16, tag="proj",
                                 name=f"ptr{ch}")
                for qt in range(NT):
                    nc.tensor.transpose(
                        ptr[:, qt, :],
                        ao_t[:, qt, :, :].rearrange("p a b -> p (a b)"),
                        ident[:])
                nc.vector.tensor_scalar_mul(
                    out=aoT[:, ch, :],
                    in0=ptr[:].rearrange("p a b -> p (a b)"), scalar1=1.0)

            # k/q projections for chunk 0 (burst)
            emit_ebh(0, pebt)
            emit_ebh(1, pebt)
            for u in (make_proj_units("k", 0, "act")
                      + make_proj_units("q", 0, "act")):
                u()
            # preload the Exp ACT table while the PE runs the bursts above
            nc.scalar.activation(out=scratch[:], in_=eps_t[:], func=AF.Exp,
                                 scale=1.0)

            # ---------------- head loop ----------------
            eh_prev = None     # eh of head 2c-1
            pas_prev = None
            spill = None       # attnv spill of head 2c-2 (qt 6,7)
            ao_cur = None

            for c in range(KC):
                h0, h1 = 2 * c, 2 * c + 1
                emit_ebh(h0 + 2, pebt)
                if c == 5:
                    dma_w(wo16, "wo")
                if c == 6:
                    nc.gpsimd.dma_start(out=bo_b[:], in_=bass.AP(
                        tensor=bo_ap.tensor, offset=bo_ap.offset,
                        ap=[[0, 128]] + list(bo_ap.ap)))
                ao_last, ao_cur = ao_cur, pao.tile([128, NT, 2, 64], f16,
                                                   tag="ao", name=f"ao{c}")

                eh0 = pc.tile([128, NT, N], f16, tag="et", name=f"eh{h0}")
                if c == 0:
                    units_k = make_proj_units("k", 1) + make_proj_units("q", 1)
                    k_fill = [2, 2, 2, 2, 2, 2, 2, 2]
                elif c < KC - 1:
                    units_k = make_proj_units("k", c + 1)
                    k_fill = [2, 1, 1, 1, 1, 1, 1, 0]
                else:
                    units_k, k_fill = [], [0] * 8
                # --- h0 phase: 8 slots ---
                for kt in range(NT):
                    if kt > 0:
                        scores_slot(h0, kt, eh0)
                    for _ in range(k_fill[kt]):
                        if units_k:
                            units_k.pop(0)()
                    if kt == 0:
                        scores_slot(h0, kt, eh0)
                    # spill: finish attnv of head 2c-2 (qt 6,7)
                    if spill is not None and kt < len(spill[4]):
                        sh, seh, spas, sao, qts = spill
                        attnv_part(sh, qts[kt], seh, spas)
                        if qts[kt] == NT - 1:
                            emit_norm(sh, spas, sao)
                    # attnv of head 2c-1, qt 0..5 on slots 2..7
                    if eh_prev is not None and kt >= 2:
                        if kt == 2:
                            pas_prev = pas_pool.tile([128, N], f32, tag="as",
                                                     name=f"pas{h0 - 1}")
                        attnv_part(h0 - 1, kt - 2, eh_prev, pas_prev)
                spill = None

                emit_ebh(h1 + 2, pebt)
                eh1 = pc.tile([128, NT, N], f16, tag="et", name=f"eh{h1}")
                units_q = make_proj_units("q", c + 1) if 0 < c < KC - 1 else []
                q_fill = [2, 1, 1, 1, 1, 1, 1, 0]
                # --- h1 phase: 8 slots ---
                for kt in range(NT):
                    if kt > 0:
                        scores_slot(h1, kt, eh1)
                    for _ in range(q_fill[kt]):
                        if units_q:
                            units_q.pop(0)()
                    if kt == 0:
                        scores_slot(h1, kt, eh1)
                    if eh_prev is not None and kt < 2:
                        # finish attnv of head 2c-1 (qt 6,7)
                        attnv_part(h0 - 1, 6 + kt, eh_prev, pas_prev)
                        if kt == 1:
                            emit_norm(h0 - 1, pas_prev, ao_last)
                            kq_tiles.pop(("k", c - 1), None)
                            kq_tiles.pop(("q", c - 1), None)
                    if kt == 2 and c >= 1:
                        emit_transpose(c - 1, ao_last)
                    # attnv of head 2c, qt 0..5 on slots 2..7
                    if kt >= 2:
                        if kt == 2:
                            pas0 = pas_pool.tile([128, N], f32, tag="as",
                                                 name=f"pas{h0}")
                        attnv_part(h0, kt - 2, eh0, pas0)
                if c < KC - 1:
                    spill = (h0, eh0, pas0, ao_cur, (6, 7))
                else:
                    for qt in (6, 7):
                        attnv_part(h0, qt, eh0, pas0)
                    emit_norm(h0, pas0, ao_cur)
                eh_prev, pas_prev = eh1, None

            # ---------------- epilogue ----------------
            def oproj_mm(fo, m, nh, kc):
                nc.tensor.matmul(
                    fo[:, nh * 512:(nh + 1) * 512],
                    aoT[:, kc, m * 128:(m + 1) * 128],
                    wo16[:, kc, nh * 512:(nh + 1) * 512],
                    start=(kc == 0), stop=(kc == KC - 1))

            def oproj_out(fo, so, m, nh):
                nh_sl = slice(nh * 512, (nh + 1) * 512)
                nc.vector.tensor_add(out=so[:, nh_sl], in0=fo[:, nh_sl],
                                     in1=bo_b[:, nh_sl])
                nc.sync.dma_start(out=out_d[m * 128:(m + 1) * 128, nh_sl],
                                  in_=so[:, nh_sl])

            # attnv + normalize for head 15
            pas15 = pas_pool.tile([128, N], f32, tag="as", name="pas15")
            for qt in range(NT):
                attnv_part(H - 1, qt, eh_prev, pas15)
            emit_norm(H - 1, pas15, ao_cur)
            emit_transpose(KC - 1, ao_cur)

            for m in range(NT):
                if m % 2 == 1:
                    fo = pas_pool.tile([128, N], f32, tag="as", name=f"fo{m}")
                else:
                    fo = pacc.tile([128, N], f32, tag="acc", name=f"fo{m}")
                so = pc.tile([128, N], f16, tag="so", name=f"so{m}")
                for nh in range(2):
                    for kc in range(KC):
                        oproj_mm(fo, m, nh, kc)
                    oproj_out(fo, so, m, nh)


def build():
    nc = bacc.Bacc()
    ins = {}
    for nm in ("x8h", "x8l", "c8h", "c8l",
               "wq8h", "wq8l", "wk8h", "wk8l", "wv8h", "wv8l"):
        ins[nm] = nc.declare_dram_parameter(nm, [D, D], f8, isOutput=False)
    ins["wo"] = nc.declare_dram_parameter("wo", [D, D], f16, isOutput=False)
    ins["bqkv"] = nc.declare_dram_parameter("bqkv", [3, D], f32,
                                            isOutput=False)
    ins["bo"] = nc.declare_dram_parameter("bo", [D], f32, isOutput=False)
    ins["ebt"] = nc.declare_dram_parameter("ebt", [H, N, N], f16,
                                           isOutput=False)
    out_d = nc.declare_dram_parameter("out", [N, D], f16, isOutput=True)
    with tile.TileContext(nc) as tc:
        _body(tc, nc, ins, out_d)
    nc.compile()
    return nc


_NC_CACHE = None


def _get_nc():
    global _NC_CACHE
    if _NC_CACHE is None:
        _NC_CACHE = build()
    return _NC_CACHE


def _hi_lo(a):
    hi = a.astype(F8NP)
    lo = (a - hi.astype(np.float32)).astype(F8NP)
    return np.ascontiguousarray(hi), np.ascontiguousarray(lo)


def _ln0(a):
    mu = a.mean(-1, keepdims=True)
    var = ((a - mu) ** 2).mean(-1, keepdims=True)
    return (a - mu) / np.sqrt(var + EPS)


def kernel(x, context, relative_position_bias, Wq, Wk, Wv, Wo, bo, gamma,
           beta):
    x = np.asarray(x, np.float32)
    context = np.asarray(context, np.float32)
    rpb = np.asarray(relative_position_bias, np.float32)
    Wq = np.asarray(Wq, np.float32)
    Wk = np.asarray(Wk, np.float32)
    Wv = np.asarray(Wv, np.float32)
    Wo = np.asarray(Wo, np.float32)
    bo = np.asarray(bo, np.float32)
    gamma = np.asarray(gamma, np.float32)
    beta = np.asarray(beta, np.float32)

    shared = {"wo": Wo.astype(np.float16),
              "bqkv": WS * np.stack([beta @ Wq, beta @ Wk,
                                     beta @ Wv]).astype(np.float32),
              "bo": bo,
              "ebt": np.exp(rpb).transpose(0, 2, 1).astype(np.float16).copy()}
    for nm, W in (("wq", Wq), ("wk", Wk), ("wv", Wv)):
        hi, lo = _hi_lo(WS * gamma[:, None] * W)
        shared[nm + "8h"], shared[nm + "8l"] = hi, lo

    in_maps = []
    for i in range(N_CORES):
        m = dict(shared)
        xh, xl = _hi_lo(_ln0(x[i]).T)
        ch, cl = _hi_lo(_ln0(context[i]).T)
        m["x8h"], m["x8l"], m["c8h"], m["c8l"] = xh, xl, ch, cl
        in_maps.append(m)

    nc = _get_nc()
    last_err = None
    for _attempt in range(3):
        try:
            res = run_bass_kernel_spmd(nc, in_maps, list(range(N_CORES)))
            break
        except Exception as e:  # transient NRT/axon exec errors
            last_err = e
    else:
        raise last_err
    return np.stack([res.results[i]["out"].astype(np.float32)
                     for i in range(N_CORES)])


# revision 49
# speedup vs baseline: 1.0085x; 1.0085x over previous
"""8-core Trainium2 (Bass/Tile) kernel for nn_CrossAttention.

Sharding: pure data parallelism - batch B=8, one batch element per
NeuronCore. Each core runs QKV projections, 16-head attention with
relative position bias, and the output projection for its element;
the host gathers the 8 outputs.

Host-side prep (cheap numpy transforms of the inputs, same category as
the exp(rpb) bias transform):
  - LayerNorm of x/context on host (fp32), shipped TRANSPOSED and
    decomposed into fp8e4 hi+lo pairs (hi = fp8(xn), lo = fp8(xn-hi)).
  - gamma folded into W{q,k,v}; weights scaled by 32 (so their fp8 lo
    parts stay above the fp8 subnormal cutoff) and decomposed hi+lo.
    The 32x cancels: scores get SM_SCALE/1024 inside the exp, and the
    attn@v rowsum column uses 32.0 instead of 1.0.
  - relative_position_bias shipped as exp(b)^T in fp16.
  - beta@W biases scaled by 32 (fp32), Wo in fp16.

Q/K/V projections run as compensated fp8 DoubleRow matmuls: three
passes (hi*hi + hi*lo + lo*hi) of K=256 contractions at 0.5 cycles per
output row - 25% fewer charged PE rows than fp16 at full precision
(the dropped lo*lo term is ~0.04%).

attn@v is "flipped": lhsT = eh tile ([128 k, 128 q]), rhs = per-head V
([128 k, 64 d]) -> psum [128 q, 64 d] accumulated over 8 kt steps, so
per head it streams 4k rows instead of 8k. Rowsums come from extra N=1
matmuls against the 32.0 column; normalization is an fp32 reciprocal +
stride-0 broadcast DVE multiply; PE transposes restore [hd, q] for the
fp16 output projection.

The head loop is software-pipelined at kt-slot granularity: each slot
is one scores matmul pair plus filler PE work (next chunk's k/q
projection subunits, the trailing head's attn@v parts, ao transposes)
so the PE tracks the ACT exp pace (~1.04us per [128,1024] tile).

PSUM (8 banks): scores 2x[128,1024]f32 (4), k/q proj halves 2x1,
attnv data+rowsums [128,1024]f32 (2).
"""

import numpy as np
import ml_dtypes

import concourse.bass as bass
import concourse.bacc as bacc
import concourse.tile as tile
from concourse import mybir
from concourse.masks import make_identity
from concourse.bass_utils import run_bass_kernel_spmd

f32 = mybir.dt.float32
f16 = mybir.dt.float16
f8 = mybir.dt.float8e4
AF = mybir.ActivationFunctionType
ALU = mybir.AluOpType
DR = mybir.MatmulPerfMode.DoubleRow

N = 1024
D = 1024
H = 16
NT = 8
KC = 8
EPS = 1e-5
WS = 32.0                      # host weight scale
SM_SCALE = 0.125 / (WS * WS)   # exp(SM_SCALE * q'k'), q'k' = 1024 qk
N_CORES = 8
F8NP = ml_dtypes.float8_e4m3fn


def _body(tc, nc, ins, out_d):
    with (
        tc.tile_pool(name="consts", bufs=1) as consts,
        tc.tile_pool(name="big", bufs=1) as big,
        tc.tile_pool(name="pacc", bufs=2, space="PSUM") as pacc,
        tc.tile_pool(name="pproj", bufs=2, space="PSUM") as pproj,
        tc.tile_pool(name="pas", bufs=1, space="PSUM") as pas_pool,
    ):
        ident = consts.tile([128, 128], f16)
        make_identity(nc, ident[:])
        eps_t = consts.tile([128, 1], f32)
        nc.vector.memset(eps_t[:], EPS)
        scratch = consts.tile([128, 1], f32)
        bq_t = consts.tile([128, KC], f32)
        bk_t = consts.tile([128, KC], f32)
        bv_ap = ins["bqkv"][2, :]
        bv_b = consts.tile([128, D], f16)
        nc.gpsimd.dma_start(out=bv_b[:], in_=bass.AP(
            tensor=bv_ap.tensor, offset=bv_ap.offset,
            ap=[[0, 128]] + list(bv_ap.ap)))
        bo_ap = ins["bo"][:]
        bo_b = consts.tile([128, D], f16)

        v_aug = big.tile([128, NT, H, 65], f16)
        aoT = big.tile([128, KC, N], f16)
        x8h = big.tile([128, KC, N], f8)
        x8l = big.tile([128, KC, N], f8)
        c8h = big.tile([128, KC, N], f8)
        c8l = big.tile([128, KC, N], f8)
        wq8h = big.tile([128, KC, D], f8)
        wq8l = big.tile([128, KC, D], f8)
        wk8h = big.tile([128, KC, D], f8)
        wk8l = big.tile([128, KC, D], f8)
        wo16 = big.tile([128, KC, D], f16)
        nc.gpsimd.memset(v_aug[:, :, :, 0:1], WS)
        v_flat = v_aug[:].rearrange("p a h c -> p a (h c)")

        def dma_w(dst, name):
            nc.sync.dma_start(out=dst[:], in_=ins[name].rearrange(
                "(a p) m -> p a m", p=128))

        ebh_tiles = {}

        def emit_ebh(h, pebt):
            if h >= H:
                return
            for half in range(2):
                t = pebt.tile([128, 4, N], f16, tag="ebt")
                nc.sync.dma_start(
                    out=t[:],
                    in_=ins["ebt"][h, half * 512:(half + 1) * 512, :].rearrange(
                        "(a p) q -> p a q", p=128))
                ebh_tiles[(h, half)] = t

        # ---------------- prologue ----------------
        with tc.tile_pool(name="pwv", bufs=1) as pwv:
            wv8h = pwv.tile([128, KC, D], f8)
            wv8l = pwv.tile([128, KC, D], f8)
            # DMA issue order: feed vproj's hh pass first, then the rest
            dma_w(c8h, "c8h")
            dma_w(wv8h, "wv8h")
            dma_w(c8l, "c8l")
            dma_w(wv8l, "wv8l")
            nc.sync.dma_start(out=bq_t[:], in_=ins["bqkv"][0, :].rearrange(
                "(m p) -> p m", p=128))
            nc.sync.dma_start(out=bk_t[:], in_=ins["bqkv"][1, :].rearrange(
                "(m p) -> p m", p=128))
            dma_w(wk8h, "wk8h")
            dma_w(wk8l, "wk8l")
            dma_w(x8h, "x8h")
            dma_w(x8l, "x8l")
            dma_w(wq8h, "wq8h")
            dma_w(wq8l, "wq8l")

            def emit_vproj(t):
                pv = pacc.tile([128, N], f32, tag="acc")
                for nq in range(4):
                    idx = 0
                    for (cs, ws) in ((c8h, wv8h), (c8l, wv8h), (c8h, wv8l)):
                        for a in range(4):
                            nc.tensor.matmul(
                                pv[:, nq * 256:(nq + 1) * 256],
                                cs[:, 2 * a:2 * a + 2, t * 128:(t + 1) * 128],
                                ws[:, 2 * a:2 * a + 2,
                                   nq * 256:(nq + 1) * 256],
                                start=(idx == 0), stop=(idx == 11),
                                perf_mode=DR)
                            idx += 1
                nc.vector.tensor_add(
                    out=v_aug[:, t, :, 1:65],
                    in0=pv[:].rearrange("p (h d) -> p h d", h=H),
                    in1=bv_b[:].rearrange("p (h d) -> p h d", h=H))

            for t in range(NT):
                emit_vproj(t)

        with tc.tile_pool(name="pkq", bufs=4) as pkq, \
             tc.tile_pool(name="pc", bufs=3) as pc, \
             tc.tile_pool(name="pebt", bufs=3) as pebt, \
             tc.tile_pool(name="pao", bufs=2) as pao, \
             tc.tile_pool(name="prec", bufs=2) as prec:

            kq_tiles = {}
            WX = {"k": (wk8h, wk8l, c8h, c8l), "q": (wq8h, wq8l, x8h, x8l)}

            def proj_sub(which, c, half, nq, sub):
                """6 of the 12 DoubleRow matmuls of one 256-tok quarter."""
                key = (which, c, half)
                if key not in kq_tiles:
                    kq_tiles[key] = pproj.tile([128, 512], f32, tag="proj",
                                               name=f"p{which}{c}h{half}")
                ph = kq_tiles[key]
                wh, wl, xh, xl = WX[which]
                mms = [(w, x, a) for (w, x) in ((wh, xh), (wh, xl), (wl, xh))
                       for a in range(4)]
                tok0 = half * 512 + nq * 256
                for i in range(6 * sub, 6 * sub + 6):
                    w, x, a = mms[i]
                    nc.tensor.matmul(
                        ph[:, nq * 256:(nq + 1) * 256],
                        w[:, 2 * a:2 * a + 2, c * 128:(c + 1) * 128],
                        x[:, 2 * a:2 * a + 2, tok0:tok0 + 256],
                        start=(i == 0), stop=(i == 11), perf_mode=DR)

            def proj_bias(which, c, half, eng="dve"):
                ph = kq_tiles.pop((which, c, half))
                dkey = (which, c)
                if dkey not in kq_tiles:
                    kq_tiles[dkey] = pkq.tile([128, N], f16, tag="kq",
                                              name=f"{which}T{c}")
                bias = bk_t if which == "k" else bq_t
                dst = kq_tiles[dkey][:, half * 512:(half + 1) * 512]
                if eng == "act":
                    nc.scalar.add(out=dst, in_=ph[:], add=bias[:, c:c + 1])
                else:
                    nc.vector.tensor_scalar(
                        out=dst, in0=ph[:], scalar1=bias[:, c:c + 1],
                        scalar2=None, op0=ALU.add)

            def make_proj_units(which, c, bias_eng="dve"):
                units = []
                for half in range(2):
                    for nq in range(2):
                        for sub in range(2):
                            def u(which=which, c=c, half=half, nq=nq,
                                  sub=sub):
                                proj_sub(which, c, half, nq, sub)
                                if nq == 1 and sub == 1:
                                    proj_bias(which, c, half, bias_eng)
                            units.append(u)
                return units

            def scores_slot(h, kt, eh_t):
                """One kt tile of scores + exp + ebt multiply."""
                ch, r0 = h // 2, (h % 2) * 64
                kTc = kq_tiles[("k", ch)]
                qTc = kq_tiles[("q", ch)]
                ps = pacc.tile([128, N], f32, tag="acc")
                for nh in range(2):
                    nc.tensor.matmul(
                        ps[:, nh * 512:(nh + 1) * 512],
                        kTc[r0:r0 + 64, kt * 128:(kt + 1) * 128],
                        qTc[r0:r0 + 64, nh * 512:(nh + 1) * 512],
                        start=True, stop=True)
                nc.scalar.activation(out=eh_t[:, kt, :], in_=ps[:],
                                     func=AF.Exp, scale=SM_SCALE)
                eng = nc.gpsimd if (kt in (3, 6) and h < H - 2) else nc.vector
                eng.tensor_mul(out=eh_t[:, kt, :], in0=eh_t[:, kt, :],
                               in1=ebh_tiles[(h, kt // 4)][:, kt % 4, :])

            def attnv_part(h, qt, eh_t, pas):
                """attn@v for one qt block of head h: 8 data + 8 sum mms."""
                for kt in range(NT):
                    nc.tensor.matmul(
                        pas[:, qt * 64:(qt + 1) * 64],
                        eh_t[:, kt, qt * 128:(qt + 1) * 128],
                        v_flat[:, kt, h * 65 + 1:h * 65 + 65],
                        start=(kt == 0), stop=(kt == NT - 1))
                for kt in range(NT):
                    nc.tensor.matmul(
                        pas[:, 512 + qt:513 + qt],
                        eh_t[:, kt, qt * 128:(qt + 1) * 128],
                        v_flat[:, kt, h * 65:h * 65 + 1],
                        start=(kt == 0), stop=(kt == NT - 1))

            def emit_norm(h, pas, ao_t):
                rec = prec.tile([128, 8], f32, tag="rec")
                nc.vector.reciprocal(out=rec[:], in_=pas[:, 512:520])
                rec_b = bass.AP(tensor=rec.tensor, offset=rec.offset,
                                ap=[[8, 128], [1, 8], [0, 64]])
                nc.vector.tensor_mul(
                    out=ao_t[:, :, h % 2, :],
                    in0=pas[:, 0:512].rearrange("p (a b) -> p a b", a=8),
                    in1=rec_b)

            def emit_transpose(ch, ao_t):
                ptr = pproj.tile([128, NT, 128], f16, tag="proj",
                                 name=f"ptr{ch}")
                for qt in range(NT):
                    nc.tensor.transpose(
                        ptr[:, qt, :],
                        ao_t[:, qt, :, :].rearrange("p a b -> p (a b)"),
                        ident[:])
                nc.vector.tensor_scalar_mul(
                    out=aoT[:, ch, :],
                    in0=ptr[:].rearrange("p a b -> p (a b)"), scalar1=1.0)

            # k/q projections for chunk 0 (burst)
            emit_ebh(0, pebt)
            emit_ebh(1, pebt)
            for u in (make_proj_units("k", 0, "act")
                      + make_proj_units("q", 0, "act")):
                u()
            # preload the Exp ACT table while the PE runs the bursts above
            nc.scalar.activation(out=scratch[:], in_=eps_t[:], func=AF.Exp,
                                 scale=1.0)

            # ---------------- head loop ----------------
            eh_prev = None     # eh of head 2c-1
            pas_prev = None
            spill = None       # attnv spill of head 2c-2 (qt 6,7)
            ao_cur = None

            for c in range(KC):
                h0, h1 = 2 * c, 2 * c + 1
                emit_ebh(h0 + 2, pebt)
                if c == 5:
                    dma_w(wo16, "wo")
                if c == 6:
                    nc.gpsimd.dma_start(out=bo_b[:], in_=bass.AP(
                        tensor=bo_ap.tensor, offset=bo_ap.offset,
                        ap=[[0, 128]] + list(bo_ap.ap)))
                ao_last, ao_cur = ao_cur, pao.tile([128, NT, 2, 64], f16,
                                                   tag="ao", name=f"ao{c}")

                eh0 = pc.tile([128, NT, N], f16, tag="et", name=f"eh{h0}")
                if c == 0:
                    units_k = make_proj_units("k", 1) + make_proj_units("q", 1)
                    k_fill = [2, 2, 2, 2, 2, 2, 2, 2]
                elif c < KC - 1:
                    units_k = make_proj_units("k", c + 1)
                    k_fill = [2, 2, 1, 1, 1, 1, 0, 0]
                else:
                    units_k, k_fill = [], [0] * 8
                # --- h0 phase: 8 slots ---
                for kt in range(NT):
                    if kt > 0:
                        scores_slot(h0, kt, eh0)
                    for _ in range(k_fill[kt]):
                        if units_k:
                            units_k.pop(0)()
                    if kt == 0:
                        scores_slot(h0, kt, eh0)
                    # spill: finish attnv of head 2c-2 (qt 6,7)
                    if spill is not None and kt < len(spill[4]):
                        sh, seh, spas, sao, qts = spill
                        attnv_part(sh, qts[kt], seh, spas)
                        if qts[kt] == NT - 1:
                            emit_norm(sh, spas, sao)
                    # attnv of head 2c-1, qt 0..5 on slots 2..7
                    if eh_prev is not None and kt >= 2:
                        if kt == 2:
                            pas_prev = pas_pool.tile([128, N], f32, tag="as",
                                                     name=f"pas{h0 - 1}")
                        attnv_part(h0 - 1, kt - 2, eh_prev, pas_prev)
                spill = None

                emit_ebh(h1 + 2, pebt)
                eh1 = pc.tile([128, NT, N], f16, tag="et", name=f"eh{h1}")
                units_q = make_proj_units("q", c + 1) if 0 < c < KC - 1 else []
                q_fill = [2, 2, 1, 1, 1, 1, 0, 0]
                # --- h1 phase: 8 slots ---
                for kt in range(NT):
                    if kt > 0:
                        scores_slot(h1, kt, eh1)
                    for _ in range(q_fill[kt]):
                        if units_q:
                            units_q.pop(0)()
                    if kt == 0:
                        scores_slot(h1, kt, eh1)
                    if eh_prev is not None and kt < 2:
                        # finish attnv of head 2c-1 (qt 6,7)
                        attnv_part(h0 - 1, 6 + kt, eh_prev, pas_prev)
                        if kt == 1:
                            emit_norm(h0 - 1, pas_prev, ao_last)
                            kq_tiles.pop(("k", c - 1), None)
                            kq_tiles.pop(("q", c - 1), None)
                    if kt == 2 and c >= 1:
                        emit_transpose(c - 1, ao_last)
                    # attnv of head 2c, qt 0..5 on slots 2..7
                    if kt >= 2:
                        if kt == 2:
                            pas0 = pas_pool.tile([128, N], f32, tag="as",
                                                 name=f"pas{h0}")
                        attnv_part(h0, kt - 2, eh0, pas0)
                if c < KC - 1:
                    spill = (h0, eh0, pas0, ao_cur, (6, 7))
                else:
                    for qt in (6, 7):
                        attnv_part(h0, qt, eh0, pas0)
                    emit_norm(h0, pas0, ao_cur)
                eh_prev, pas_prev = eh1, None

            # ---------------- epilogue ----------------
            def oproj_mm(fo, m, nh, kc):
                nc.tensor.matmul(
                    fo[:, nh * 512:(nh + 1) * 512],
                    aoT[:, kc, m * 128:(m + 1) * 128],
                    wo16[:, kc, nh * 512:(nh + 1) * 512],
                    start=(kc == 0), stop=(kc == KC - 1))

            def oproj_out(fo, so, m, nh):
                nh_sl = slice(nh * 512, (nh + 1) * 512)
                nc.vector.tensor_add(out=so[:, nh_sl], in0=fo[:, nh_sl],
                                     in1=bo_b[:, nh_sl])
                nc.sync.dma_start(out=out_d[m * 128:(m + 1) * 128, nh_sl],
                                  in_=so[:, nh_sl])

            # attnv + normalize for head 15
            pas15 = pas_pool.tile([128, N], f32, tag="as", name="pas15")
            for qt in range(NT):
                attnv_part(H - 1, qt, eh_prev, pas15)
            emit_norm(H - 1, pas15, ao_cur)
            emit_transpose(KC - 1, ao_cur)

            for m in range(NT):
                if m % 2 == 1:
                    fo = pas_pool.tile([128, N], f32, tag="as", name=f"fo{m}")
                else:
                    fo = pacc.tile([128, N], f32, tag="acc", name=f"fo{m}")
                so = pc.tile([128, N], f16, tag="so", name=f"so{m}")
                for nh in range(2):
                    for kc in range(KC):
                        oproj_mm(fo, m, nh, kc)
                    oproj_out(fo, so, m, nh)


def build():
    nc = bacc.Bacc()
    ins = {}
    for nm in ("x8h", "x8l", "c8h", "c8l",
               "wq8h", "wq8l", "wk8h", "wk8l", "wv8h", "wv8l"):
        ins[nm] = nc.declare_dram_parameter(nm, [D, D], f8, isOutput=False)
    ins["wo"] = nc.declare_dram_parameter("wo", [D, D], f16, isOutput=False)
    ins["bqkv"] = nc.declare_dram_parameter("bqkv", [3, D], f32,
                                            isOutput=False)
    ins["bo"] = nc.declare_dram_parameter("bo", [D], f32, isOutput=False)
    ins["ebt"] = nc.declare_dram_parameter("ebt", [H, N, N], f16,
                                           isOutput=False)
    out_d = nc.declare_dram_parameter("out", [N, D], f16, isOutput=True)
    with tile.TileContext(nc) as tc:
        _body(tc, nc, ins, out_d)
    nc.compile()
    return nc


_NC_CACHE = None


def _get_nc():
    global _NC_CACHE
    if _NC_CACHE is None:
        _NC_CACHE = build()
    return _NC_CACHE


def _hi_lo(a):
    hi = a.astype(F8NP)
    lo = (a - hi.astype(np.float32)).astype(F8NP)
    return np.ascontiguousarray(hi), np.ascontiguousarray(lo)


def _ln0(a):
    mu = a.mean(-1, keepdims=True)
    var = ((a - mu) ** 2).mean(-1, keepdims=True)
    return (a - mu) / np.sqrt(var + EPS)


def kernel(x, context, relative_position_bias, Wq, Wk, Wv, Wo, bo, gamma,
           beta):
    x = np.asarray(x, np.float32)
    context = np.asarray(context, np.float32)
    rpb = np.asarray(relative_position_bias, np.float32)
    Wq = np.asarray(Wq, np.float32)
    Wk = np.asarray(Wk, np.float32)
    Wv = np.asarray(Wv, np.float32)
    Wo = np.asarray(Wo, np.float32)
    bo = np.asarray(bo, np.float32)
    gamma = np.asarray(gamma, np.float32)
    beta = np.asarray(beta, np.float32)

    shared = {"wo": Wo.astype(np.float16),
              "bqkv": WS * np.stack([beta @ Wq, beta @ Wk,
                                     beta @ Wv]).astype(np.float32),
              "bo": bo,
              "ebt": np.exp(rpb).transpose(0, 2, 1).astype(np.float16).copy()}
    for nm, W in (("wq", Wq), ("wk", Wk), ("wv", Wv)):
        hi, lo = _hi_lo(WS * gamma[:, None] * W)
        shared[nm + "8h"], shared[nm + "8l"] = hi, lo

    in_maps = []
    for i in range(N_CORES):
        m = dict(shared)
        xh, xl = _hi_lo(_ln0(x[i]).T)
        ch, cl = _hi_lo(_ln0(context[i]).T)
        m["x8h"], m["x8l"], m["c8h"], m["c8l"] = xh, xl, ch, cl
        in_maps.append(m)

    nc = _get_nc()
    last_err = None
    for _attempt in range(3):
        try:
            res = run_bass_kernel_spmd(nc, in_maps, list(range(N_CORES)))
            break
        except Exception as e:  # transient NRT/axon exec errors
            last_err = e
    else:
        raise last_err
    return np.stack([res.results[i]["out"].astype(np.float32)
                     for i in range(N_CORES)])
